# revision 3
# baseline (speedup 1.0000x reference)
"""Position Attention Module (DANet) on 8 Trainium2 NeuronCores.

Reference computation (per batch b of 4):
  xf = x[b] : [C=512, N=4096]
  q = Wq@xf + bq : [64, N];  k = Wk@xf + bk : [64, N];  v = Wv@xf + bv : [512, N]
  scores[i,j] = q[:,i].k[:,j];  attn = softmax_j(scores)
  out[c,i] = alpha * sum_j v[c,j] attn[i,j]

Sharding: 2 cores per batch, each core owns half the query rows (i), full k/v.
Per-core x is pre-rolled on host so the owned i-half is always columns 0:2048.

Device layout choices:
  - scoresT [j, i] (lhsT=k, rhs=q): softmax denominator handled by DVE
    accumulation + ones-matmul; avoids transposing attn for the AV matmul.
  - exp without max-subtraction: scores ~ N(0,64); |s|max ~ 56 << 88 (fp32 safe).
  - out [c, i] directly: lhsT = vT[j,c] chunks (vT produced transposed by the
    projection, using host-transposed Wv), rhs = expT [j, i].
  - all matmuls in float32r (TF32-ish, 1 cycle/row) accumulating fp32 in PSUM.
"""
import numpy as np

B, C, HW = 4, 512, 4096
CQ = 64
NCORES = 8
IH = HW // 2          # 2048 query rows per core
ITILE = 512           # i-tile (psum free dim)
NITILES = IH // ITILE # 4
JT = 128              # j-tile (contraction chunk for AV / scores lhsT cols)
NJT = HW // JT        # 32
JB = 512              # j-block for projections
NJB = HW // JB        # 8
NCC = C // 128        # 4 contraction chunks of 128 over C

_cache = {}


def _build():
    import concourse.bacc as bacc
    import concourse.tile as tile
    import concourse.mybir as mybir
    from concourse.bass_utils import run_bass_kernel_spmd

    f32 = mybir.dt.float32
    f32r = mybir.dt.float32r
    AF = mybir.ActivationFunctionType

    nc = bacc.Bacc("TRN2", target_bir_lowering=False, debug=False)

    x_d = nc.dram_tensor("x", [C, HW], f32, kind="ExternalInput")
    wqt_d = nc.dram_tensor("wqt", [C, CQ], f32, kind="ExternalInput")
    wkt_d = nc.dram_tensor("wkt", [C, CQ], f32, kind="ExternalInput")
    wvt_d = nc.dram_tensor("wvt", [C, C], f32, kind="ExternalInput")
    bq_d = nc.dram_tensor("bq", [CQ, 1], f32, kind="ExternalInput")
    bk_d = nc.dram_tensor("bk", [CQ, 1], f32, kind="ExternalInput")
    bv_d = nc.dram_tensor("bv", [1, C], f32, kind="ExternalInput")
    al_d = nc.dram_tensor("alpha", [1, 1], f32, kind="ExternalInput")
    out_d = nc.dram_tensor("out", [C, IH], f32, kind="ExternalOutput")

    with tile.TileContext(nc) as tc:
        with (
            tc.tile_pool(name="const", bufs=1) as cpool,
            tc.tile_pool(name="kq", bufs=1) as kqpool,
            tc.tile_pool(name="vt", bufs=1) as vtpool,
        ):
            # --- constants / weights ---
            wqt = [cpool.tile([128, CQ], f32r, tag=f"wqt{i}", name=f"wqt{i}") for i in range(NCC)]
            wkt = [cpool.tile([128, CQ], f32r, tag=f"wkt{i}", name=f"wkt{i}") for i in range(NCC)]
            wvt = [cpool.tile([128, C], f32r, tag=f"wvt{i}", name=f"wvt{i}") for i in range(NCC)]
            for cc in range(NCC):
                sl = slice(cc * 128, (cc + 1) * 128)
                nc.sync.dma_start(wqt[cc][:], wqt_d[sl, :].bitcast(f32r))
                nc.sync.dma_start(wkt[cc][:], wkt_d[sl, :].bitcast(f32r))
                nc.sync.dma_start(wvt[cc][:], wvt_d[sl, :].bitcast(f32r))
            bq_c = cpool.tile([CQ, 1], f32, tag="bqc")
            bk_c = cpool.tile([CQ, 1], f32, tag="bkc")
            nc.sync.dma_start(bq_c[:], bq_d[:])
            nc.sync.dma_start(bk_c[:], bk_d[:])
            bv_row = cpool.tile([1, C], f32, tag="bvrow")
            nc.sync.dma_start(bv_row[:], bv_d[:])
            al_sb = cpool.tile([1, 1], f32, tag="alsb")
            nc.sync.dma_start(al_sb[:], al_d[:])
            ones_r = cpool.tile([1, 128], f32, tag="onesr")   # K=1 bcast lhsT
            nc.vector.memset(ones_r[:], 1.0)
            ones_c = cpool.tile([128, 1], f32, tag="onesc")   # partition-sum lhsT
            nc.vector.memset(ones_c[:], 1.0)

            # k [64, HW], q [64, IH] activations; vT as 32 tiles [128 j, C]
            k_sb = kqpool.tile([CQ, HW], f32r, tag="ksb")
            q_sb = kqpool.tile([CQ, IH], f32r, tag="qsb")
            vts = [vtpool.tile([JT, C], f32r, tag=f"vt{j}", name=f"vt{j}") for j in range(NJT)]

            # bvB: bv broadcast to 128 partitions (for vT psum eviction)
            with tc.tile_pool(name="ppre", bufs=1, space="PSUM") as ppre:
                bvB = cpool.tile([128, C], f32, tag="bvB")
                ps = ppre.tile([128, C], f32, tag="bvps")
                nc.tensor.matmul(ps[:], ones_r[:], bv_row[:], start=True, stop=True)
                nc.vector.tensor_copy(bvB[:], ps[:])

            # ---------------- projections ----------------
            with (
                tc.tile_pool(name="xin", bufs=8) as xpool,
                tc.tile_pool(name="pkq", bufs=2, space="PSUM") as pkq,
                tc.tile_pool(name="pvt", bufs=3, space="PSUM") as pvt,
            ):
                for jb in range(NJB):
                    jsl = slice(jb * JB, (jb + 1) * JB)
                    xt = []
                    for cc in range(NCC):
                        t = xpool.tile([128, JB], f32r, tag="x")
                        nc.sync.dma_start(
                            t[:], x_d[cc * 128:(cc + 1) * 128, jsl].bitcast(f32r)
                        )
                        xt.append(t)
                    # k (and q for the owned half) : [64, JB]
                    kp = pkq.tile([CQ, JB], f32, tag="kqp")
                    for cc in range(NCC):
                        nc.tensor.matmul(kp[:], wkt[cc][:], xt[cc][:],
                                         start=(cc == 0), stop=(cc == NCC - 1))
                    nc.scalar.activation(k_sb[:, jsl], kp[:], AF.Identity, bias=bk_c[:])
                    if jb < NJB // 2:
                        qp = pkq.tile([CQ, JB], f32, tag="kqp")
                        for cc in range(NCC):
                            nc.tensor.matmul(qp[:], wqt[cc][:], xt[cc][:],
                                             start=(cc == 0), stop=(cc == NCC - 1))
                        nc.scalar.activation(q_sb[:, jsl], qp[:], AF.Identity,
                                             bias=bq_c[:])
                    # vT tiles [128 j, C]
                    for js in range(JB // JT):
                        vp = pvt.tile([JT, C], f32, tag="vtp")
                        for cc in range(NCC):
                            nc.tensor.matmul(
                                vp[:], xt[cc][:, js * JT:(js + 1) * JT], wvt[cc][:],
                                start=(cc == 0), stop=(cc == NCC - 1))
                        nc.vector.tensor_add(vts[jb * 4 + js][:], vp[:], bvB[:])

            # ---------------- attention ----------------
            with (
                tc.tile_pool(name="expp", bufs=3) as epool,
                tc.tile_pool(name="dnm", bufs=2) as dpool,
                tc.tile_pool(name="ost", bufs=8) as opool,
                tc.tile_pool(name="rows", bufs=2) as rpool,
                tc.tile_pool(name="pso", bufs=2, space="PSUM") as pso,
                tc.tile_pool(name="pout", bufs=4, space="PSUM") as pout,
                tc.tile_pool(name="paux", bufs=1, space="PSUM") as paux,
            ):
                for it in range(NITILES):
                    isl = slice(it * ITILE, (it + 1) * ITILE)
                    ops = [pout.tile([128, ITILE], f32, tag="op", name=f"op{it}_{i}") for i in range(NCC)]
                    dnm = dpool.tile([128, ITILE], f32, tag="dn")
                    for j in range(NJT):
                        sp = pso.tile([JT, ITILE], f32, tag="sc")
                        nc.tensor.matmul(sp[:], k_sb[:, j * JT:(j + 1) * JT],
                                         q_sb[:, isl], start=True, stop=True)
                        et = epool.tile([JT, ITILE], f32r, tag="exp")
                        nc.scalar.activation(et[:], sp[:], AF.Exp)
                        if j == 0:
                            nc.vector.tensor_copy(dnm[:], et[:])
                        else:
                            nc.vector.tensor_add(dnm[:], dnm[:], et[:])
                        for cc in range(NCC):
                            nc.tensor.matmul(
                                ops[cc][:], vts[j][:, cc * 128:(cc + 1) * 128], et[:],
                                start=(j == 0), stop=(j == NJT - 1))
                    # scale = alpha / denom  (row [1, ITILE]), broadcast to 128
                    drow = paux.tile([1, ITILE], f32, tag="aux")
                    nc.tensor.matmul(drow[:], ones_c[:], dnm[:], start=True, stop=True)
                    rrow = rpool.tile([1, ITILE], f32, tag="rrow")
                    nc.vector.reciprocal(rrow[:], drow[:])
                    srow = rpool.tile([1, ITILE], f32, tag="srow")
                    nc.vector.tensor_scalar_mul(srow[:], rrow[:], al_sb[:])
                    sB = paux.tile([128, ITILE], f32, tag="aux")
                    nc.tensor.matmul(sB[:], ones_r[:], srow[:], start=True, stop=True)
                    sB_sb = rpool.tile([128, ITILE], f32, tag="sbsb")
                    nc.vector.tensor_copy(sB_sb[:], sB[:])
                    for cc in range(NCC):
                        ot = opool.tile([128, ITILE], f32, tag="ot")
                        nc.vector.tensor_mul(ot[:], ops[cc][:], sB_sb[:])
                        nc.sync.dma_start(out_d[cc * 128:(cc + 1) * 128, isl], ot[:])

    nc.compile()
    return nc, run_bass_kernel_spmd


def kernel(x, Wq, bq, Wk, bk, Wv, bv, alpha, trace=False, trace_kwargs=None):
    if "nc" not in _cache:
        _cache["nc"] = _build()
    nc, run_spmd = _cache["nc"]

    x = np.ascontiguousarray(np.asarray(x, dtype=np.float32)).reshape(B, C, HW)
    wqt = np.ascontiguousarray(np.asarray(Wq, np.float32).T)
    wkt = np.ascontiguousarray(np.asarray(Wk, np.float32).T)
    wvt = np.ascontiguousarray(np.asarray(Wv, np.float32).T)
    bq = np.asarray(bq, np.float32).reshape(CQ, 1)
    bk = np.asarray(bk, np.float32).reshape(CQ, 1)
    bv = np.asarray(bv, np.float32).reshape(1, C)
    alpha = np.asarray(alpha, np.float32).reshape(1, 1)

    in_maps = []
    for core in range(NCORES):
        b, ih = core // 2, core % 2
        xb = x[b]
        if ih:
            xb = np.ascontiguousarray(np.concatenate([xb[:, IH:], xb[:, :IH]], axis=1))
        in_maps.append({"x": xb, "wqt": wqt, "wkt": wkt, "wvt": wvt,
                        "bq": bq, "bk": bk, "bv": bv, "alpha": alpha})

    kwargs = {}
    if trace:
        kwargs["trace"] = True
        kwargs.update(trace_kwargs or {})
    res = run_spmd(nc, in_maps, list(range(NCORES)), **kwargs)

    out = np.empty((B, C, HW), dtype=np.float32)
    for core in range(NCORES):
        b, ih = core // 2, core % 2
        out[b][:, ih * IH:(ih + 1) * IH] = res.results[core]["out"]
    if trace:
        return out.reshape(B, C, 64, 64), res
    return out.reshape(B, C, 64, 64)
